# revision 1
# baseline (speedup 1.0000x reference)
"""Trainium2 Bass kernel for nn_MoEExpertPool (MoE product-of-experts).

Math (per reference):
  3 modality groups (fs, cb, sp) x 4 experts each = 12 experts.
  Per expert e: h = relu(x @ W1_e + b1_e); mu_e = h @ Wmu_e + bmu_e;
                lv_e = h @ Wlv_e + blv_e.
  Gate per group: w = softmax(x @ Wg + bg) (cb additionally scaled by
  (1 - mean(modality_mask))).  PoE fuse over the 12 experts:
    prec_e = 1 / (exp(lv_e) + eps)
    S2 = sum_e prec_e ; S1 = sum_e w_e * mu_e * prec_e
    mu_fused = S1 / S2 ; logvar_fused = log(1/S2 + eps)

Sharding: pure batch-parallel over 8 cores (512 rows each); every core runs
all 12 experts so S1/S2 are complete per-core — no cross-core reduction.
Device works in a transposed layout (contraction dim on partitions), so
matmuls chain with no on-chip transposes and the per-column biases become
per-partition activation biases.  Matmul operands are float32r: fp32 data
through the PE at 1 cycle/row (4x faster than plain fp32; ~2.3e-4 rel err).
Gates + final divide/log are computed on host (0.01% of FLOPs).
"""

import os
import sys

sys.path.insert(0, "/opt/trn_rl_repo")

import numpy as np

B, D, E, NG = 4096, 2048, 4, 3
NEXP = NG * E            # 12 experts
N_CORES = 8
BC = B // N_CORES        # 512 batch rows per core
MT = D // 128            # 16 output tiles per matmul
KT = D // 128            # 16 contraction tiles
EPS = 1e-8

USE_BF16 = os.environ.get("KERNEL_BF16", "0") == "1"
WSTRIP_BUFS = 8 if USE_BF16 else 6
H_BUFS = 1

_cache = {}


def _bcol(e, j, mt):
    # column in the packed bias tile for expert e, matrix j (0=b1,1=bmu,2=blv)
    return (e * 3 + j) * MT + mt


def _build_nc(reps=1):
    import concourse.mybir as mybir
    import concourse.tile as tile
    from concourse import bacc

    f32 = mybir.dt.float32
    # Matmul operand dtype.  float32r must be declared end-to-end (DRAM and
    # SBUF): the BIR verifier rejects bitcasts into fp32r matmuls, and only
    # gpsimd DMA may cast.  float32r is byte-identical to f32 on the numpy
    # side; the PE rounds internally (~2.3e-4 rel err at K=2048).
    mmdt = mybir.dt.bfloat16 if USE_BF16 else mybir.dt.float32r
    dramdt = mmdt
    AF = mybir.ActivationFunctionType

    nc = bacc.Bacc("TRN2", target_bir_lowering=False)
    xT = nc.dram_tensor("xT", [D, BC], dramdt, kind="ExternalInput")
    W = nc.dram_tensor("W", [NEXP * 3, D, D], dramdt, kind="ExternalInput")
    WG = nc.dram_tensor("WG", [NEXP, BC], f32, kind="ExternalInput")
    BIAS = nc.dram_tensor("BIAS", [128, NEXP * 3 * MT], f32, kind="ExternalInput")
    S1 = nc.dram_tensor("S1", [D, BC], f32, kind="ExternalOutput")
    S2 = nc.dram_tensor("S2", [D, BC], f32, kind="ExternalOutput")

    with tile.TileContext(nc) as tc:
        with (
            tc.tile_pool(name="xp", bufs=1) as xp,
            tc.tile_pool(name="hp", bufs=H_BUFS) as hp,
            tc.tile_pool(name="accp", bufs=1) as accp,
            tc.tile_pool(name="wp", bufs=WSTRIP_BUFS) as wp,
            tc.tile_pool(name="gp", bufs=2) as gp,
            tc.tile_pool(name="cp", bufs=1) as cp,
            tc.tile_pool(name="ew", bufs=3) as ew,
            tc.tile_pool(name="psh", bufs=2, space="PSUM") as psh,
            tc.tile_pool(name="psmu", bufs=3, space="PSUM") as psmu,
            tc.tile_pool(name="pslv", bufs=3, space="PSUM") as pslv,
        ):
            xsb = xp.tile([128, KT, BC], mmdt)
            nc.sync.dma_start(xsb[:], xT[:, :].rearrange("(kt p) b -> p kt b", p=128))
            bias_sb = cp.tile([128, NEXP * 3 * MT], f32)
            nc.sync.dma_start(bias_sb[:], BIAS[:, :])
            S1sb = accp.tile([128, MT, BC], f32)
            S2sb = accp.tile([128, MT, BC], f32)

            # reps>1 builds a timing variant that repeats the whole
            # computation; only the last rep's outputs are stored.
            for rep in range(reps):
              nc.vector.memset(S1sb[:], 0.0)
              nc.vector.memset(S2sb[:], 0.0)
              for e in range(NEXP):
                  wg_t = gp.tile([128, BC], f32, tag="wg")
                  nc.sync.dma_start(wg_t[:], WG[e : e + 1, :].partition_broadcast(128))

                  h = hp.tile([128, KT, BC], mmdt, tag="h")
                  # layer 1: hT = relu(W1.T @ xT + b1)
                  for mt in range(MT):
                      wst = wp.tile([128, KT, 128], mmdt, tag="wstrip")
                      nc.sync.dma_start(
                          wst[:],
                          W[3 * e, :, mt * 128 : (mt + 1) * 128].rearrange(
                              "(kt p) m -> p kt m", p=128
                          ),
                      )
                      ps = psh.tile([128, BC], f32, tag="psh")
                      for kt in range(KT):
                          nc.tensor.matmul(
                              ps[:],
                              wst[:, kt, :],
                              xsb[:, kt, :],
                              start=(kt == 0),
                              stop=(kt == KT - 1),
                          )
                      nc.scalar.activation(
                          h[:, mt, :], ps[:], AF.Relu,
                          bias=bias_sb[:, _bcol(e, 0, mt) : _bcol(e, 0, mt) + 1],
                      )
                  # layer 2: muT, lvT; fold into PoE partial sums
                  for mt in range(MT):
                      wmu = wp.tile([128, KT, 128], mmdt, tag="wstrip")
                      nc.sync.dma_start(
                          wmu[:],
                          W[3 * e + 1, :, mt * 128 : (mt + 1) * 128].rearrange(
                              "(kt p) m -> p kt m", p=128
                          ),
                      )
                      wlv = wp.tile([128, KT, 128], mmdt, tag="wstrip")
                      nc.sync.dma_start(
                          wlv[:],
                          W[3 * e + 2, :, mt * 128 : (mt + 1) * 128].rearrange(
                              "(kt p) m -> p kt m", p=128
                          ),
                      )
                      pmu = psmu.tile([128, BC], f32, tag="pmu")
                      plv = pslv.tile([128, BC], f32, tag="plv")
                      for kt in range(KT):
                          nc.tensor.matmul(
                              pmu[:], wmu[:, kt, :], h[:, kt, :],
                              start=(kt == 0), stop=(kt == KT - 1),
                          )
                      for kt in range(KT):
                          nc.tensor.matmul(
                              plv[:], wlv[:, kt, :], h[:, kt, :],
                              start=(kt == 0), stop=(kt == KT - 1),
                          )
                      elv = ew.tile([128, BC], f32, tag="elv")
                      nc.scalar.activation(
                          elv[:], plv[:], AF.Exp,
                          bias=bias_sb[:, _bcol(e, 2, mt) : _bcol(e, 2, mt) + 1],
                      )
                      nc.vector.tensor_scalar_add(elv[:], elv[:], EPS)
                      prec = ew.tile([128, BC], f32, tag="prec")
                      nc.vector.reciprocal(prec[:], elv[:])
                      nc.vector.tensor_add(S2sb[:, mt, :], S2sb[:, mt, :], prec[:])
                      mu = ew.tile([128, BC], f32, tag="mu")
                      nc.vector.scalar_tensor_tensor(
                          mu[:], pmu[:],
                          bias_sb[:, _bcol(e, 1, mt) : _bcol(e, 1, mt) + 1],
                          prec[:],
                          op0=mybir.AluOpType.add, op1=mybir.AluOpType.mult,
                      )
                      nc.vector.tensor_mul(mu[:], mu[:], wg_t[:])
                      nc.vector.tensor_add(S1sb[:, mt, :], S1sb[:, mt, :], mu[:])

            for mt in range(MT):
                nc.sync.dma_start(S1[mt * 128 : (mt + 1) * 128, :], S1sb[:, mt, :])
                nc.sync.dma_start(S2[mt * 128 : (mt + 1) * 128, :], S2sb[:, mt, :])

    nc.compile()
    return nc


def _get_nc(reps=1):
    key = ("nc", reps)
    if key not in _cache:
        _cache[key] = _build_nc(reps)
    return _cache[key]


def _host_prep(inputs):
    x = np.asarray(inputs["x"], np.float32)
    mask = np.asarray(inputs["modality_mask"])
    xd = x.astype(np.float64)
    mask_mean = mask.astype(np.float64).mean(axis=1, keepdims=True)  # [B,1]

    if USE_BF16:
        import ml_dtypes
        mmdt_np = ml_dtypes.bfloat16
    else:
        mmdt_np = np.float32

    prefs = ["fs", "cb", "sp"]
    # gate weights [NEXP, B]
    wgate = np.empty((NEXP, B), np.float32)
    for g, pref in enumerate(prefs):
        logits = xd @ np.asarray(inputs[f"{pref}_Wg"], np.float64) + np.asarray(
            inputs[f"{pref}_bg"], np.float64
        )
        logits -= logits.max(axis=1, keepdims=True)
        ex = np.exp(logits)
        w = ex / ex.sum(axis=1, keepdims=True)  # [B, E]
        if pref == "cb":
            w = w * (1.0 - mask_mean)
        wgate[g * E : (g + 1) * E, :] = w.T.astype(np.float32)

    Wstack = np.empty((NEXP * 3, D, D), mmdt_np)
    bias_arr = np.zeros((128, NEXP * 3 * MT), np.float32)
    for g, pref in enumerate(prefs):
        for e in range(E):
            ge = g * E + e
            for j, nm in enumerate(["W1", "Wmu", "Wlv"]):
                Wstack[ge * 3 + j] = np.asarray(inputs[f"{pref}_{nm}"][e]).astype(
                    mmdt_np
                )
            for j, nm in enumerate(["b1", "bmu", "blv"]):
                vec = np.asarray(inputs[f"{pref}_{nm}"][e], np.float32)  # [D]
                bias_arr[:, (ge * 3 + j) * MT : (ge * 3 + j + 1) * MT] = vec.reshape(
                    MT, 128
                ).T

    xt = np.ascontiguousarray(x.T.astype(mmdt_np))  # [D, B]
    in_maps = []
    for c in range(N_CORES):
        in_maps.append(
            {
                "xT": np.ascontiguousarray(xt[:, c * BC : (c + 1) * BC]),
                "W": Wstack,
                "WG": np.ascontiguousarray(wgate[:, c * BC : (c + 1) * BC]),
                "BIAS": bias_arr,
            }
        )
    return in_maps


def _finalize(results):
    S1 = np.concatenate([r["S1"] for r in results], axis=1)  # [D, B]
    S2 = np.concatenate([r["S2"] for r in results], axis=1)  # [D, B]
    S2d = S2.astype(np.float64)
    mu_fused = (S1.astype(np.float64) / S2d).T.astype(np.float32)
    logvar_fused = np.log(1.0 / S2d + EPS).T.astype(np.float32)
    return mu_fused, logvar_fused


def kernel(run_kwargs=None, **inputs):
    from concourse.bass_utils import run_bass_kernel_spmd

    nc = _get_nc()
    in_maps = _host_prep(inputs)
    res = run_bass_kernel_spmd(
        nc, in_maps, core_ids=list(range(N_CORES)), **(run_kwargs or {})
    )
    _cache["last_result"] = res
    return _finalize(res.results)



# revision 2
# speedup vs baseline: 1.0513x; 1.0513x over previous
"""Trainium2 Bass kernel for nn_MoEExpertPool (MoE product-of-experts), v2.

Math (per reference):
  3 modality groups (fs, cb, sp) x 4 experts each = 12 experts.
  Per expert e: h = relu(x @ W1_e + b1_e); mu_e = h @ Wmu_e + bmu_e;
                lv_e = h @ Wlv_e + blv_e.
  Gate per group: w = softmax(x @ Wg + bg) (cb scaled by (1-mean(mask))).
  PoE fuse: prec_e = 1/(exp(lv_e)+eps); S2 = sum_e prec_e;
            S1 = sum_e w_e*mu_e*prec_e; mu = S1/S2; logvar = log(1/S2+eps).

Sharding v2: (expert-triple x batch-half).  Core c gets batch half c//4
(2048 tokens) and experts {3*(c%4)..3*(c%4)+2}.  Device layout is
transposed (contraction on partitions); weights are pre-rearranged on
host so every strip DMA is one contiguous 4KB line per partition.
Cores write per-expert partial S1/S2 ([d, token] f32) straight to DRAM
(no big SBUF accumulators) and the host does the 24-way reduce + final
divide/log in f64.

Matmuls are bf16 (1 cycle/row, ~4e-3 rel err end-to-end vs 2e-2 gate).
prec = exp(-(plv+blv)) on the scalar engine (scale=-1) — skips the
reference's +eps (relative effect <=1e-6) and the DVE reciprocal.
Gates + final math on host (0.01% of FLOPs).
"""

import sys

sys.path.insert(0, "/opt/trn_rl_repo")

import numpy as np

B, D, E, NG = 4096, 2048, 4, 3
NEXP = NG * E            # 12 experts
N_CORES = 8
EPT = 3                  # experts per core (triple)
W = 2048                 # tokens per core (batch half)
NCHUNK = W // 512        # moving chunks per stationary tile
MT = D // 128            # 16 output tiles
KT = D // 128            # 16 contraction tiles
EPS = 1e-8

_cache = {}


def _bcol(ei, j, mt):
    # column in the packed bias tile for expert-slot ei, matrix j
    # (j: 0=b1, 1=bmu, 2=-blv)
    return (ei * 3 + j) * MT + mt


def _build_nc():
    import concourse.mybir as mybir
    import concourse.tile as tile
    from concourse import bacc

    f32 = mybir.dt.float32
    bf = mybir.dt.bfloat16
    AF = mybir.ActivationFunctionType

    nc = bacc.Bacc("TRN2", target_bir_lowering=False)
    # Host-side layouts pre-rearranged for contiguous DMA lines:
    #  XT[p, kt, w]            = x_half.T[kt*128+p, w]
    #  WS[m, mt, p, kt*128+k]  = W_mat[m][kt*128+p, mt*128+k]  (m = ei*3+j)
    XT = nc.dram_tensor("XT", [128, KT, W], bf, kind="ExternalInput")
    WS = nc.dram_tensor("WS", [EPT * 3, MT, 128, KT * 128], bf, kind="ExternalInput")
    WG = nc.dram_tensor("WG", [EPT, W], f32, kind="ExternalInput")
    BIAS = nc.dram_tensor("BIAS", [128, EPT * 3 * MT], f32, kind="ExternalInput")
    PS1 = nc.dram_tensor("PS1", [EPT, D, W], f32, kind="ExternalOutput")
    PS2 = nc.dram_tensor("PS2", [EPT, D, W], f32, kind="ExternalOutput")

    with tile.TileContext(nc) as tc:
        with (
            tc.tile_pool(name="xp", bufs=1) as xp,
            tc.tile_pool(name="hp", bufs=1) as hp,
            tc.tile_pool(name="wp", bufs=4) as wp,
            tc.tile_pool(name="gp", bufs=1) as gp,
            tc.tile_pool(name="cp", bufs=1) as cp,
            tc.tile_pool(name="ew", bufs=2) as ew,
            tc.tile_pool(name="psp", bufs=2, space="PSUM") as psp,
        ):
            xsb = xp.tile([128, KT, W], bf)
            nc.sync.dma_start(xsb[:], XT[:])
            bias_sb = cp.tile([128, EPT * 3 * MT], f32)
            nc.sync.dma_start(bias_sb[:], BIAS[:, :])

            def bank(ch):
                # independent 1-bank PSUM tiles (bufs=2 x 4 tags = 8 banks)
                # so WAR on a bank only waits for that bank's own drain.
                return psp.tile([128, 512], f32, tag=f"pb{ch}", name=f"pb{ch}")

            for ei in range(EPT):
                wg_t = gp.tile([128, W], f32, tag="wg")
                nc.sync.dma_start(
                    wg_t[:], WG[ei : ei + 1, :].partition_broadcast(128)
                )

                h = hp.tile([128, KT, W], bf, tag="h")
                # layer 1: hT = relu(W1.T @ xT + b1); per-chunk chains+drain
                for mt in range(MT):
                    wst = wp.tile([128, KT, 128], bf, tag="wstrip")
                    nc.sync.dma_start(
                        wst[:],
                        WS[3 * ei, mt].rearrange("p (kt k) -> p kt k", kt=KT),
                    )
                    for ch in range(NCHUNK):
                        cs = slice(ch * 512, (ch + 1) * 512)
                        ps = bank(ch)
                        for kt in range(KT):
                            nc.tensor.matmul(
                                ps[:], wst[:, kt, :], xsb[:, kt, cs],
                                start=(kt == 0), stop=(kt == KT - 1),
                            )
                        nc.scalar.activation(
                            h[:, mt, cs], ps[:], AF.Relu,
                            bias=bias_sb[:, _bcol(ei, 0, mt) : _bcol(ei, 0, mt) + 1],
                        )
                # layer 2: muT, lvT; emit per-expert PoE partials.
                # mu/lv chains interleave at chunk level; each bank drains
                # while the next chunk's chains run.
                for mt in range(MT):
                    wmu = wp.tile([128, KT, 128], bf, tag="wstrip")
                    nc.sync.dma_start(
                        wmu[:],
                        WS[3 * ei + 1, mt].rearrange("p (kt k) -> p kt k", kt=KT),
                    )
                    wlv = wp.tile([128, KT, 128], bf, tag="wstrip")
                    nc.sync.dma_start(
                        wlv[:],
                        WS[3 * ei + 2, mt].rearrange("p (kt k) -> p kt k", kt=KT),
                    )
                    prec = ew.tile([128, W], f32, tag="prec")
                    mu = ew.tile([128, W], f32, tag="mu")
                    s1 = ew.tile([128, W], f32, tag="s1")
                    for ch in range(NCHUNK):
                        cs = slice(ch * 512, (ch + 1) * 512)
                        pmu = bank(ch)
                        for kt in range(KT):
                            nc.tensor.matmul(
                                pmu[:], wmu[:, kt, :], h[:, kt, cs],
                                start=(kt == 0), stop=(kt == KT - 1),
                            )
                        plv = bank(ch)
                        for kt in range(KT):
                            nc.tensor.matmul(
                                plv[:], wlv[:, kt, :], h[:, kt, cs],
                                start=(kt == 0), stop=(kt == KT - 1),
                            )
                        # prec = exp(-(plv + blv)); S2 partial = prec
                        nc.scalar.activation(
                            prec[:, cs], plv[:], AF.Exp,
                            bias=bias_sb[:, _bcol(ei, 2, mt) : _bcol(ei, 2, mt) + 1],
                            scale=-1.0,
                        )
                        # S1 partial = (pmu + bmu) * prec * wg
                        nc.vector.scalar_tensor_tensor(
                            mu[:, cs], pmu[:],
                            bias_sb[:, _bcol(ei, 1, mt) : _bcol(ei, 1, mt) + 1],
                            prec[:, cs],
                            op0=mybir.AluOpType.add,
                            op1=mybir.AluOpType.mult,
                        )
                        nc.vector.tensor_mul(s1[:, cs], mu[:, cs], wg_t[:, cs])
                    nc.sync.dma_start(
                        PS2[ei, mt * 128 : (mt + 1) * 128, :], prec[:]
                    )
                    nc.sync.dma_start(
                        PS1[ei, mt * 128 : (mt + 1) * 128, :], s1[:]
                    )

    nc.compile()
    return nc


def _get_nc(**_ignored):
    if "nc" not in _cache:
        _cache["nc"] = _build_nc()
    return _cache["nc"]


def _host_prep(inputs):
    import ml_dtypes

    bf = ml_dtypes.bfloat16
    x = np.asarray(inputs["x"], np.float32)
    mask = np.asarray(inputs["modality_mask"])
    xd = x.astype(np.float64)
    mask_mean = mask.astype(np.float64).mean(axis=1, keepdims=True)  # [B,1]

    prefs = ["fs", "cb", "sp"]
    # gate weights [NEXP, B] (cb gates pre-scaled by (1 - mask_mean))
    wgate = np.empty((NEXP, B), np.float32)
    for g, pref in enumerate(prefs):
        logits = xd @ np.asarray(inputs[f"{pref}_Wg"], np.float64) + np.asarray(
            inputs[f"{pref}_bg"], np.float64
        )
        logits -= logits.max(axis=1, keepdims=True)
        ex = np.exp(logits)
        w = ex / ex.sum(axis=1, keepdims=True)  # [B, E]
        if pref == "cb":
            w = w * (1.0 - mask_mean)
        wgate[g * E : (g + 1) * E, :] = w.T.astype(np.float32)

    # xt_r[kt, p, b] = x.T[kt*128+p, b]
    xt_r = np.ascontiguousarray(x.T).astype(bf).reshape(KT, 128, B)

    in_maps = []
    for c in range(N_CORES):
        half = c // 4
        experts = [3 * (c % 4) + i for i in range(EPT)]  # global expert ids

        ws = np.empty((EPT * 3, MT, 128, KT * 128), bf)
        bias_arr = np.zeros((128, EPT * 3 * MT), np.float32)
        wg = np.empty((EPT, W), np.float32)
        for ei, ge in enumerate(experts):
            g, e = ge // E, ge % E
            pref = prefs[g]
            for j, nm in enumerate(["W1", "Wmu", "Wlv"]):
                wmat = np.asarray(inputs[f"{pref}_{nm}"][e], np.float32)  # [D, D]
                # [kt,p, mt,k] -> [mt, p, kt, k]
                ws[ei * 3 + j] = (
                    wmat.astype(bf)
                    .reshape(KT, 128, MT, 128)
                    .transpose(2, 1, 0, 3)
                    .reshape(MT, 128, KT * 128)
                )
            for j, nm in enumerate(["b1", "bmu", "blv"]):
                vec = np.asarray(inputs[f"{pref}_{nm}"][e], np.float32)  # [D]
                if nm == "blv":
                    vec = -vec  # prec = exp(-(plv + blv))
                bias_arr[:, (ei * 3 + j) * MT : (ei * 3 + j + 1) * MT] = (
                    vec.reshape(MT, 128).T
                )
            wg[ei] = wgate[ge, half * W : (half + 1) * W]

        in_maps.append(
            {
                "XT": np.ascontiguousarray(
                    xt_r[:, :, half * W : (half + 1) * W].transpose(1, 0, 2)
                ),
                "WS": ws,
                "WG": wg,
                "BIAS": bias_arr,
            }
        )
    return in_maps


def _finalize(results):
    # results[c]["PS1"/"PS2"]: [EPT, D, W] f32 partials; cores 0-3 cover
    # half 0, cores 4-7 half 1; sum partials per half over (core, expert).
    S1 = np.zeros((D, B), np.float64)
    S2 = np.zeros((D, B), np.float64)
    for c, r in enumerate(results):
        half = c // 4
        sl = slice(half * W, (half + 1) * W)
        S1[:, sl] += r["PS1"].astype(np.float64).sum(axis=0)
        S2[:, sl] += r["PS2"].astype(np.float64).sum(axis=0)
    mu_fused = (S1 / S2).T.astype(np.float32)
    logvar_fused = np.log(1.0 / S2 + EPS).T.astype(np.float32)
    return mu_fused, logvar_fused


def kernel(run_kwargs=None, **inputs):
    from concourse.bass_utils import run_bass_kernel_spmd

    nc = _get_nc()
    in_maps = _host_prep(inputs)
    res = run_bass_kernel_spmd(
        nc, in_maps, core_ids=list(range(N_CORES)), **(run_kwargs or {})
    )
    _cache["last_result"] = res
    return _finalize(res.results)
